# revision 85
# baseline (speedup 1.0000x reference)
"""Trainium2 Bass kernel: full 1-D convolution (2,097,152 samples x 32,000-tap
RIR) + peak-normalize, via FFT overlap-save, distributed over 8 NeuronCores.

Each core processes 3 overlap-save blocks of M = 131072 = D1*D2 (D1=256,
D2=512).  A block is convolved as y = IDFT(DFT(x) * Hhat) with both DFTs done
as two-stage matmul FFTs on the PE array (n = n1 + 256*n2, k = k2 + 512*k1):

  s1: S[k2,n1]   = sum_n2 F512[n2,k2] x[n1+256*n2]        (real input, k2<=256)
  t1: B = S^T * exp(-2pi i n1 k2 / M)                      (DVE twiddle)
  s3: X[k1,k2]   = sum_n1 F256[n1,k1] B[n1,k2]             (k1 < 128 half)
  pw: Y = X * Hhat[k1,k2]                                  (DVE pointwise)
  s4: D[k2,n1]   = sum_k1 Y[k1,k2] G256[n1,k1]             (k2 in 1..256 ONLY)
  t2: E = D * exp(+2pi i n1 k2 / M)/1024                   (DVE twiddle, half)
  s6: y[n2,n1]   = sum_{k2=1..256} 2 Re(G512[n2,k2] E)     (doubled weights)

The inverse side exploits E[n1,.] being the DFT of the real row y[n1,.]:
E is Hermitian along k2, so S4/S6 run on k2 in [1,256] with doubled S6
weights.  The k1 in [128,256) completion uses Yf[j'] = Y[:, 511-j'] flipped
copies + sign-folded tables.  The DC column (k2=0) is computed by two tiny
[1,256] matmuls and packed into the (otherwise zero-weighted) imag plane of
the k2=256 row; the S6 weight row there is 1s.  The (k1=128, k2=0) M/2 bin
is dropped (same as the f32 reference's error floor; ~1e-3 relative).

Data path fp16 (PE matmuls full rate, DVE tensor ops 2x); PSUM fp32.
Peak-normalize: per-block |max| reduce, AllReduce(max) across 8 cores,
scale by 1/max(m, 1) before the store.
"""
import numpy as np

M = 131072
D1 = 256                  # n1 / k1 extent
D2 = 512                  # n2 / k2 extent
KLEN = 32_000             # RIR taps
N = 2_097_152             # signal samples
NOUT = N + KLEN - 1       # full-convolution output length
U = 90112                 # kept samples per block (352 rows of 256)
DROWS = 160               # discarded n2-rows per block (160*256 = 40960 >= K-1)
KROWS = 352               # kept n2-rows per block
NBLK = 3                  # blocks per core
NCORES = 8
HSCALE = 1024.0           # folded into Hhat; 1/HSCALE folded into tw2/dc

_NC_CACHE = None
STAGE_MARKS = []


def _build_nc(collective=True, passes=1, debug_taps=False):
    import concourse.bacc as bacc
    import concourse.bass as bass
    import concourse.mybir as mybir
    from concourse import tile

    f32 = mybir.dt.float32
    f16 = mybir.dt.float16
    OP = mybir.AluOpType
    AX = mybir.AxisListType.X

    nc = bacc.Bacc("TRN2", target_bir_lowering=False, debug=False,
                   num_devices=NCORES)

    x_in = nc.dram_tensor("x", [NBLK, 128, 4 * D1], f16, kind="ExternalInput")
    wf512 = nc.dram_tensor("wf512", [128, 2056], f16, kind="ExternalInput")
    wf256 = nc.dram_tensor("wf256", [128, 768], f16, kind="ExternalInput")
    wg256 = nc.dram_tensor("wg256", [128, 1536], f16, kind="ExternalInput")
    wg512 = nc.dram_tensor("wg512", [128, 2048], f16, kind="ExternalInput")
    tw1_in = nc.dram_tensor("tw1", [128, 1028], f16, kind="ExternalInput")
    tw2_in = nc.dram_tensor("tw2", [128, 1024], f16, kind="ExternalInput")
    hh_in = nc.dram_tensor("hh", [128, 1024], f16, kind="ExternalInput")
    dc_in = nc.dram_tensor("dc", [128, 512], f16, kind="ExternalInput")
    wf256c_in = nc.dram_tensor("wf256c", [128, 768], f16,
                               kind="ExternalInput")
    y_out = nc.dram_tensor("y", [128, 9 * D1], f16, kind="ExternalOutput")
    if debug_taps:
        dbg = {n: nc.dram_tensor(f"dbg_{n}", [128, w], f16,
                                 kind="ExternalOutput")
               for n, w in (("ssw0", 514), ("ssw1", 514),
                            ("b0", 514), ("b1", 514),
                            ("yh0", 512), ("yh1", 512),
                            ("yf0", 256), ("yf1", 256),
                            ("dt", 1024), ("et", 1024), ("ysb", 768))}

    with tile.TileContext(nc) as tc:
        with (
            tc.tile_pool(name="wpool", bufs=1) as wpool,
            tc.tile_pool(name="data", bufs=2) as dpool,
            tc.tile_pool(name="ypool", bufs=1) as ypool,
            tc.tile_pool(name="ps", bufs=8, space="PSUM") as ps_pool,
            tc.tile_pool(name="dram", bufs=1, space="DRAM") as dram_pool,
        ):
            # ---- weight SBUF tiles ----
            wf512m = wpool.tile([128, 2056], f16, name="wf512m")
            wg512m = wpool.tile([128, 2048], f16, name="wg512m")
            wf256m = wpool.tile([128, 768], f16, name="wf256m")
            wg256m = wpool.tile([128, 1536], f16, name="wg256m")
            tw1m = wpool.tile([128, 1028], f16, name="tw1m")
            tw2m = wpool.tile([128, 1024], f16, name="tw2m")
            hhm = wpool.tile([128, 1024], f16, name="hhm")
            dcm = wpool.tile([128, 512], f16, name="dcm")
            wf256cm = wpool.tile([128, 768], f16, name="wf256cm")
            # plane views
            tw1_sb = [tw1m[:, 514 * p:514 * (p + 1)] for p in range(2)]
            hh_sb = [hhm[:, 512 * p:512 * (p + 1)] for p in range(2)]
            wf256_sb = [wf256m[:, 256 * w:256 * (w + 1)] for w in range(3)]
            wf256c_sb = [wf256cm[:, 256 * w:256 * (w + 1)] for w in range(3)]
            # s4 tables: [C, S, mS, Cp, mSp, mCp]
            g4 = [wg256m[:, 256 * w:256 * (w + 1)] for w in range(6)]
            # s6 tables: [WrA, WiA, WrB, WiB] each 512 wide
            wg512_sb = [wg512m[:, 512 * w:512 * (w + 1)] for w in range(4)]

            y_all = ypool.tile([128, 9 * D1], f16, name="y_all")
            y_sb = [y_all[:, 3 * D1 * b:3 * D1 * (b + 1)]
                    for b in range(NBLK)]
            y16 = ypool.tile([128, 9 * D1], f16, name="y16")
            mx = ypool.tile([128, 3 * NBLK], f32, name="mx")
            am = ypool.tile([128, 1], f32, name="am")
            gm = ypool.tile([128, 1], f32, name="gm")
            scb = ypool.tile([128, 1], f32, name="scb")

            # warm the PE (p-state ramp) during the initial DMA wait
            warm = ypool.tile([128, 512], f16, name="warm")
            nc.gpsimd.memset(warm[:], 0.0)
            wps = ps_pool.tile([128, 512], f32, name="wps", tag="ps")
            for _ in range(4):
                nc.tensor.matmul(wps[:], warm[:, :128], warm[:],
                                 start=True, stop=True)

            # ---- input / weight DMAs ordered by first use ----
            x_sb = [dpool.tile([128, 4 * D1], f16, name=f"x{b}", tag="x",
                               bufs=3) for b in range(NBLK)]
            nc.sync.dma_start(x_sb[0][:], x_in[0])
            for c in range(4):
                nc.sync.dma_start(wf512m[:, 514 * c:514 * (c + 1)],
                                  wf512[:, 514 * c:514 * (c + 1)])
                if c == 0:
                    nc.sync.dma_start(x_sb[1][:], x_in[1])
            nc.sync.dma_start(x_sb[2][:], x_in[2])
            nc.scalar.dma_start(tw1m[:], tw1_in[:, :])
            nc.sync.dma_start(wf256m[:], wf256[:, :])
            nc.sync.dma_start(wf256cm[:], wf256c_in[:, :])
            nc.sync.dma_start(hhm[:], hh_in[:, :])
            nc.sync.dma_start(wg256m[:], wg256[:, :])
            nc.sync.dma_start(dcm[:], dc_in[:, :])
            nc.sync.dma_start(tw2m[:], tw2_in[:, :])
            nc.sync.dma_start(wg512m[:], wg512[:, :])

            def cmul(dst_r, dst_i, sr, si, tr, ti, tm, npool=1):
                # z = s * t via 4 mults + add/sub.  One product (m3) on the
                # Pool engine, issued first; the real plane (dst_r) is
                # produced early so downstream matmul chains that read it
                # first can start ~0.6us sooner.
                m0, m1, m2, m3 = tm
                if npool:
                    nc.gpsimd.tensor_mul(m3, si, tr)
                nc.vector.tensor_mul(m0, sr, tr)
                nc.vector.tensor_mul(m1, si, ti)
                nc.vector.tensor_sub(dst_r, m0, m1)
                nc.vector.tensor_mul(m2, sr, ti)
                if not npool:
                    nc.vector.tensor_mul(m3, si, tr)
                nc.vector.tensor_add(dst_i, m2, m3)

            def one_pass():
                nc.gpsimd.memset(mx[:], 0.0)

                # per-block SBUF tiles (tag rotation = double buffering)
                Ssw, B_sb, X_sb, Yh_sb, Yf_sb = [], [], [], [], []
                DtR, EtR, tmps, dcc = [], [], [], []

                def half(t, p):
                    # plane view of the [DrA DiA DrB DiB] layout: 2-run AP
                    return t[:, 0:1024].rearrange(
                        "p (c w) -> p c w", c=2)[:, :, 256 * p:256 * (p + 1)]
                for b in range(NBLK):
                    # bufs=3: the stage-grouped schedule keeps all three
                    # blocks' tiles live simultaneously
                    Ssw.append([dpool.tile([128, 514], f16,
                                name=f"Ssw{b}_{p}", tag=f"Ssw{p}", bufs=3)
                                for p in range(2)])
                    B_sb.append([dpool.tile([128, 514], f16,
                                 name=f"B{b}_{p}", tag=f"B{p}", bufs=3)
                                 for p in range(2)])
                    X_sb.append([dpool.tile([128, D2], f16,
                                 name=f"X{b}_{p}", tag=f"X{p}", bufs=3)
                                 for p in range(2)])
                    Yh_sb.append([dpool.tile([128, D2], f16,
                                  name=f"Yh{b}_{p}", tag=f"Yh{p}", bufs=3)
                                  for p in range(2)])
                    Yf_sb.append([dpool.tile([128, 256], f16,
                                  name=f"Yf{b}_{p}", tag=f"Yf{p}", bufs=3)
                                  for p in range(2)])
                    # layout [DrA(256) DiA DrB DiB]; plane views are 2-run APs
                    DtR.append(dpool.tile([128, 1024], f16, name=f"Dt{b}",
                                          tag="Dt", bufs=3))
                    EtR.append(dpool.tile([128, 1024], f16, name=f"Et{b}",
                                          tag="Et", bufs=3))

                    tmps.append({st: [dpool.tile([128, w], f16,
                                  name=f"tmp{st}{b}_{i}", tag=f"tmp{st}{i}",
                                  bufs=3)
                                  for i in range(4)]
                                 for st, w in (("1", 514), ("3", 512),
                                               ("4", 512))})

                def st_S1(b):
                    # S^T[n1,k2] = x^T @ F512 (x chunks stationary); real
                    # input.  Chunk-major for b=0 so compute starts as soon
                    # as the first weight chunk lands.
                    pss = [ps_pool.tile([128, 512], f32, tag="ps",
                                        name=f"s1_{b}_{n1h}_{p}")
                           for n1h in range(2) for p in range(2)]
                    def mm(n1h, p, c):
                        nc.tensor.matmul(
                            pss[2 * n1h + p][:, 0:257],
                            x_sb[b][:, D1 * c + 128 * n1h:
                                    D1 * c + 128 * (n1h + 1)],
                            wf512m[:, 514 * c + 257 * p:
                                   514 * c + 257 * (p + 1)],
                            start=(c == 0), stop=(c == 3))
                    if b == 0:
                        for c in range(4):
                            for n1h in range(2):
                                for p in range(2):
                                    mm(n1h, p, c)
                    else:
                        for n1h in range(2):
                            for p in range(2):
                                for c in range(4):
                                    mm(n1h, p, c)
                    for n1h in range(2):
                        for p in range(2):
                            nc.scalar.copy(
                                Ssw[b][p][:, 257 * n1h:257 * (n1h + 1)],
                                pss[2 * n1h + p][:, 0:257])
                    # twiddle: B = S^T * tw1.  Block 0 (thin PE cover)
                    # does it per-half so S3_0 can start on half 0 early.
                    if b == 0:
                        for hh_ in range(2):
                            sl = slice(257 * hh_, 257 * (hh_ + 1))
                            cmul(B_sb[b][0][:, sl], B_sb[b][1][:, sl],
                                 Ssw[b][0][:, sl], Ssw[b][1][:, sl],
                                 tw1_sb[0][:, sl], tw1_sb[1][:, sl],
                                 [t[:, sl] for t in tmps[b]["1"]], npool=1)
                    else:
                        cmul(B_sb[b][0][:], B_sb[b][1][:],
                             Ssw[b][0][:], Ssw[b][1][:],
                             tw1_sb[0][:], tw1_sb[1][:],
                             [t[:] for t in tmps[b]["1"]], npool=1)

                def st_S3(b):
                    # X[k1,k2] for k1 < 128 (Hermitian supplies the rest);
                    # then Y = X * Hhat; then flipped copies Yf[j]=Y[511-j].
                    for pout in range(2):
                        terms = ([(0, 0), (2, 1)] if pout == 0
                                 else [(1, 0), (0, 1)])
                        fterms = ([(0, 0), (1, 1)] if pout == 0
                                  else [(1, 0), (2, 1)])
                        ps = ps_pool.tile([128, 512], f32, tag="ps",
                                          name=f"s3_{b}_{pout}")
                        seq = [(w, sp, c) for c in range(2)
                               for (w, sp) in terms]
                        for i, (w, sp, c) in enumerate(seq):
                            nc.tensor.matmul(
                                ps[:, 0:256],
                                wf256_sb[w][:, 128 * c:128 * (c + 1)],
                                B_sb[b][sp][:, 257 * c:257 * c + 256],
                                start=(i == 0), stop=(i == 3))
                        # out cols 256:512 = W_c-weighted conj-flip
                        seq = [(w, sp, c) for c in range(2)
                               for (w, sp) in fterms]
                        for i, (w, sp, c) in enumerate(seq):
                            nc.tensor.matmul(
                                ps[:, 256:512],
                                wf256c_sb[w][:, 128 * c:128 * (c + 1)],
                                B_sb[b][sp][:, 257 * c + 256:
                                            257 * c:-1],
                                start=(i == 0), stop=(i == 3))
                        nc.scalar.copy(X_sb[b][pout][:], ps[:])
                    cmul(Yh_sb[b][0][:], Yh_sb[b][1][:],
                         X_sb[b][0][:], X_sb[b][1][:],
                         hh_sb[0][:], hh_sb[1][:],
                         [t[:] for t in tmps[b]["3"]], npool=1)
                    # flipped copies for the k1-completion reads in S4;
                    # the last block's go on Pool (idle late) to keep Act's
                    # endgame queue clear for the y evacuations
                    yf_eng = (nc.gpsimd.tensor_copy if b == NBLK - 1
                              else nc.scalar.copy)
                    yf_eng(Yf_sb[b][0][:], Yh_sb[b][0][:, 511:255:-1])
                    yf_eng(Yf_sb[b][1][:], Yh_sb[b][1][:, 511:255:-1])

                def st_S4(b):
                    # D[k2,n1] for k2 in 1..256 (two 128-chunks), plus the
                    # DC column via two [1,256] matmuls; then E = D * tw2.
                    yr, yi = Yh_sb[b][0], Yh_sb[b][1]
                    yfr, yfi = Yf_sb[b][0], Yf_sb[b][1]
                    pcs = []
                    for ci, (dlo, flo) in enumerate(((1, 0), (129, 128))):
                        ps = ps_pool.tile([128, 512], f32, tag="ps",
                                          name=f"s4_{b}_{ci}")
                        pcs.append(ps)
                        dsl = slice(dlo, dlo + 128)
                        fsl = slice(flo, flo + 128)
                        # Dr chain: Yr*C + Yi*(-S) + Yfr*C' + Yfi*(-S')
                        # Di chain: Yr*S + Yi*C + Yfr*(-S') + Yfi*(-C')
                        chains = [(0, [(yr, dsl, 0), (yi, dsl, 2),
                                       (yfr, fsl, 3), (yfi, fsl, 4)]),
                                  (1, [(yr, dsl, 1), (yi, dsl, 0),
                                       (yfr, fsl, 4), (yfi, fsl, 5)])]
                        for pout, chain in chains:
                            reg = ps[:, 256 * pout:256 * (pout + 1)]
                            for i, (src, sl, w) in enumerate(chain):
                                nc.tensor.matmul(
                                    reg, src[:, sl], g4[w][:],
                                    start=(i == 0), stop=(i == 3))
                    # evacuate chunk psums into [DrA DiA DrB DiB]
                    nc.scalar.copy(DtR[b][:, 0:512], pcs[0][:])
                    nc.scalar.copy(DtR[b][:, 512:1024], pcs[1][:])
                    # twiddle (single cmul over both chunks via 2-run APs)
                    cmul(half(EtR[b], 0), half(EtR[b], 1),
                         half(DtR[b], 0), half(DtR[b], 1),
                         half(tw2m, 0), half(tw2m, 1),
                         [t[:] for t in tmps[b]["4"]], npool=1)

                def st_S6(b):
                    # y[n2,n1] = sum over k2-chunks of doubled-weight real
                    # parts; n2h=0 all-discard: skip.  n2h order 1,2,3 with
                    # the |max| reduce issued right after each chain so the
                    # final reduce in the tail is a single 256-wide op.
                    if b == NBLK - 1:
                        # separate banks: the per-chain psum reduces must
                        # not block the next chain's matmuls (bank-granular
                        # read/write serialization)
                        ps_y = [ps_pool.tile([128, 512], f32, tag="ps",
                                             name=f"y{n}_{b}")
                                for n in range(3)]
                        regmap = {1: ps_y[0][:, 0:256],
                                  2: ps_y[1][:, 0:256],
                                  3: ps_y[2][:, 0:256]}
                    else:
                        ps_y = [ps_pool.tile([128, 512], f32, tag="ps",
                                             name=f"y23_{b}"),
                                ps_pool.tile([128, 512], f32, tag="ps",
                                             name=f"y1_{b}")]
                        regmap = {2: ps_y[0][:, 0:256],
                                  3: ps_y[0][:, 256:512],
                                  1: ps_y[1][:, 0:256]}
                    yr, yi = Yh_sb[b][0], Yh_sb[b][1]
                    ybc = [yr[:, 0:1].broadcast_to((128, 128)),
                           yi[:, 0:1].broadcast_to((128, 128))]
                    for n2h in (1, 2, 3):
                        reg = regmap[n2h]
                        # DC rows (replicated broadcast-lhsT matmuls), then
                        # (weight-table, Et col range): WrA*DrA + WiA*DiA
                        # + WrB*DrB + WiB*DiB
                        nc.tensor.matmul(reg, ybc[0], dcm[:, 0:256],
                                         start=True, stop=False)
                        nc.tensor.matmul(reg, ybc[1], dcm[:, 256:512],
                                         start=False, stop=False)
                        seq = [(0, 0), (2, 512), (1, 256), (3, 768)]
                        for i, (w, off) in enumerate(seq):
                            nc.tensor.matmul(
                                reg,
                                wg512_sb[w][:, 128 * n2h:128 * (n2h + 1)],
                                EtR[b][:, off:off + 256],
                                start=False, stop=(i == 3))
                        if b == NBLK - 1:
                            # last block: reduce each chain from psum
                            # immediately so the tail reduce is short
                            if n2h == 1:
                                # copy early, zero discard rows (free
                                # memset), one reduce from fp16 staging
                                nc.scalar.copy(y_sb[b][:, 512:768],
                                               regmap[1])
                                nc.vector.memset(y_sb[b][0:32, 512:768],
                                                 0.0)
                                nc.vector.tensor_reduce(
                                    mx[:, 3 * b:3 * b + 1],
                                    y_sb[b][:, 512:768],
                                    axis=AX, op=OP.max,
                                    apply_absolute_value=True)
                            else:
                                nc.vector.tensor_reduce(
                                    mx[:, 3 * b + n2h - 1:3 * b + n2h],
                                    regmap[n2h], axis=AX, op=OP.max,
                                    apply_absolute_value=True)
                        if n2h == 1 and b < NBLK - 1:
                            nc.scalar.copy(y_sb[b][:, 512:768],
                                           regmap[1])
                    if b == NBLK - 1:
                        nc.scalar.copy(y_sb[b][:, 0:256], regmap[2])
                        nc.scalar.copy(y_sb[b][:, 256:512], regmap[3])
                    else:
                        nc.scalar.copy(y_sb[b][:, 0:512], ps_y[0][:])
                    if b < NBLK - 1:
                        # earlier blocks: zero the discarded rows (memset is
                        # free in the cost model) and reduce from the fp16
                        # staging in two halves (finer DVE slots, keeps the
                        # tail backlog short)
                        nc.vector.memset(y_sb[b][0:32, 512:768], 0.0)
                        nc.vector.tensor_reduce(
                            mx[:, 3 * b:3 * b + 1], y_sb[b][:, 0:384],
                            axis=AX, op=OP.max, apply_absolute_value=True)
                        nc.vector.tensor_reduce(
                            mx[:, 3 * b + 1:3 * b + 2], y_sb[b][:, 384:768],
                            axis=AX, op=OP.max, apply_absolute_value=True)

                stage_fn = {"S1": st_S1, "S3": st_S3,
                            "S4": st_S4, "S6": st_S6}
                schedule = [("S1", 0), ("S1", 1), ("S1", 2),
                            ("S3", 0), ("S3", 1), ("S3", 2),
                            ("S4", 0), ("S4", 1), ("S6", 0),
                            ("S4", 2), ("S6", 1), ("S6", 2)]
                for st, b in schedule:
                    STAGE_MARKS.append((st, b, nc.next_id()))
                    stage_fn[st](b)
                    if debug_taps and (st, b) == ("S1", 0):
                        for p in range(2):
                            nc.sync.dma_start(dbg[f"ssw{p}"][:, :],
                                              Ssw[0][p][:])
                            nc.sync.dma_start(dbg[f"b{p}"][:, :],
                                              B_sb[0][p][:])
                    if debug_taps and (st, b) == ("S3", 0):
                        for p in range(2):
                            nc.sync.dma_start(dbg[f"yh{p}"][:, :],
                                              Yh_sb[0][p][:])
                            nc.sync.dma_start(dbg[f"yf{p}"][:, :],
                                              Yf_sb[0][p][:])
                    if debug_taps and (st, b) == ("S4", 0):
                        nc.sync.dma_start(dbg["dt"][:, :], DtR[0][:])
                        nc.sync.dma_start(dbg["et"][:, :], EtR[0][:])
                    if debug_taps and (st, b) == ("S6", 0):
                        nc.sync.dma_start(dbg["ysb"][:, :], y_sb[0][:])
                STAGE_MARKS.append(("END", 0, nc.next_id()))

                # ---- global max, scale, store ----
                nc.vector.tensor_reduce(am[:], mx[:], axis=AX, op=OP.max)
                nc.gpsimd.partition_all_reduce(
                    gm[:], am[:], 128, bass.bass_isa.ReduceOp.max)
                if collective:
                    cc_in = dram_pool.tile([128, 1], f32, name="cc_in")
                    cc_out = dram_pool.tile([128, 1], f32, name="cc_out",
                                            addr_space="Shared")
                    nc.sync.dma_start(cc_in[:], gm[:])
                    nc.gpsimd.collective_compute(
                        "AllReduce", OP.max,
                        replica_groups=[list(range(NCORES))],
                        ins=[cc_in[:].opt()], outs=[cc_out[:].opt()])
                    nc.sync.dma_start(scb[:], cc_out[:])
                    nc.vector.tensor_scalar_max(scb[:], scb[:], 1.0)
                else:
                    nc.vector.tensor_scalar_max(scb[:], gm[:], 1.0)
                nc.vector.reciprocal(scb[:], scb[:])
                # scale + store in two chunks on two DMA queues: blocks 0-1
                # are staged long before the last block, so their scale and
                # store overlap the last block's copy tail
                nc.vector.tensor_scalar_mul(y16[:, 0:1536],
                                            y_all[:, 0:1536], scb[:, 0:1])
                nc.sync.dma_start(y_out[:, 0:1536], y16[:, 0:1536])
                nc.vector.tensor_scalar_mul(y16[:, 1536:2304],
                                            y_all[:, 1536:2304], scb[:, 0:1])
                nc.gpsimd.dma_start(y_out[:, 1536:2304], y16[:, 1536:2304])

            for _ in range(passes):
                one_pass()

    nc.compile()
    return nc


def _chunk(a):
    """[R, C] -> [128, (R/128)*C]: partition-chunked SBUF layout."""
    r, c = a.shape
    return np.ascontiguousarray(
        a.reshape(r // 128, 128, c).transpose(1, 0, 2).reshape(128, -1))


def _build_tables(h):
    """fp16 host tables; returns dict of np arrays keyed by dram tensor name."""
    n1 = np.arange(D1)
    n2 = np.arange(D2)
    q = np.arange(128)
    F512 = np.exp(-2j * np.pi * np.outer(n2, n2) / D2)
    F256 = np.exp(-2j * np.pi * np.outer(n1, n1) / D1)
    t1 = np.exp(-2j * np.pi * np.outer(n1, n2) / M)
    Wc = F256 * np.exp(-2j * np.pi * n1 / D1)[:, None]
    hp = np.zeros(M)
    hp[:KLEN] = h
    H2 = (np.fft.fft(hp) * (HSCALE / M)).reshape(D1, D2)
    f = np.float16

    # s4 tables [q, n1]
    C = np.cos(2 * np.pi * np.outer(q, n1) / D1)
    S = np.sin(2 * np.pi * np.outer(q, n1) / D1)
    Cp = np.cos(2 * np.pi * np.outer(q + 1, n1) / D1)
    Sp = np.sin(2 * np.pi * np.outer(q + 1, n1) / D1)

    # dc tables [q, n1] (with /HSCALE and the doubled conj-fold; the
    # (k1=128, k2=0) M/2 bin is dropped)
    CD = np.zeros((128, D1))
    SD = np.zeros((128, D1))
    CD[0] = 1.0 / HSCALE
    CD[1:] = 2 * np.cos(2 * np.pi * np.outer(q[1:], n1) / D1) / HSCALE
    SD[1:] = -2 * np.sin(2 * np.pi * np.outer(q[1:], n1) / D1) / HSCALE

    # tw2: k2 = p+1 (chunk A) and p+129 (chunk B); [t2r(A|B) | t2i(A|B)]
    k2v = np.arange(1, 257)
    T2 = np.exp(+2j * np.pi * np.outer(k2v, n1) / M) / HSCALE  # [256, 256]
    # layout [t2rA t2iA t2rB t2iB] matching the [DrA DiA DrB DiB] tiles
    tw2t = np.concatenate([T2.real[:128], T2.imag[:128],
                           T2.real[128:], T2.imag[128:]], axis=1)

    # s6 tables [k2-row p, n2]: WrA/WiA (k2=p+1), WrB/WiB (k2=p+129);
    # row 127 of B is k2=256: weight 1, and WiB row 127 = 1s (DC slot).
    kA = np.outer(q + 1, n2)
    kB = np.outer(q + 129, n2)
    WrA = 2 * np.cos(2 * np.pi * kA / D2)
    WiA = -2 * np.sin(2 * np.pi * kA / D2)
    WrB = 2 * np.cos(2 * np.pi * kB / D2)
    WiB = -2 * np.sin(2 * np.pi * kB / D2)
    WrB[127] /= 2.0
    WiB[127] = 0.0

    # wf512: chunk-major, plane-minor: cols = 514*c + 257*p + k2
    cr, ci = _chunk(F512.real), _chunk(F512.imag)   # [128, 4*512]
    wf512m = np.empty((128, 2056))
    for c in range(4):
        wf512m[:, 514 * c:514 * c + 257] = cr[:, 512 * c:512 * c + 257]
        wf512m[:, 514 * c + 257:514 * (c + 1)] = ci[:, 512 * c:512 * c + 257]
    return {
        "wf512": wf512m.astype(f),
        "wf256": np.concatenate(
            [np.concatenate([_chunk(P)[:, 256 * c:256 * c + 128]
                             for c in range(2)], axis=1)
             for P in (F256.real, F256.imag, -F256.imag)],
            axis=1).astype(f),
        "wg256": np.concatenate([C, S, -S, Cp, -Sp, -Cp], axis=1).astype(f),
        "wg512": np.concatenate([WrA, WiA, WrB, WiB], axis=1).astype(f),
        "tw1": np.concatenate(
            [np.concatenate([_chunk(P)[:, 512 * hh_:512 * hh_ + 257]
                             for hh_ in range(2)], axis=1)
             for P in (t1.real, t1.imag)], axis=1).astype(f),
        "tw2": tw2t.astype(f),
        "hh": np.concatenate(
            [_chunk(P)[:, 0:512] for P in (H2.real, H2.imag)],
            axis=1).astype(f),
        "dc": np.concatenate([CD, SD], axis=1).astype(f),
        "wf256c": np.concatenate(
            [np.concatenate([_chunk(P)[:, 256 * c:256 * c + 128]
                             for c in range(2)], axis=1)
             for P in (Wc.real, Wc.imag, -Wc.real)],
            axis=1).astype(f),
    }


def kernel(data, rir):
    global _NC_CACHE
    from concourse.bass_utils import run_bass_kernel_spmd

    data = np.asarray(data, dtype=np.float32).reshape(-1)
    h = np.asarray(rir, dtype=np.float64).reshape(-1)

    if _NC_CACHE is None:
        _NC_CACHE = _build_nc()
    nc = _NC_CACHE

    tables = _build_tables(h)
    disc = DROWS * D1
    xp = np.zeros(disc + NCORES * NBLK * U + (M - U - disc) + 1, np.float16)
    xp[disc:disc + N] = data.astype(np.float16)
    in_maps = []
    for c in range(NCORES):
        xb = np.stack([
            _chunk(xp[U * (NBLK * c + b):U * (NBLK * c + b) + M]
                   .reshape(D2, D1))
            for b in range(NBLK)])
        in_maps.append({"x": np.ascontiguousarray(xb), **tables})
    res = run_bass_kernel_spmd(nc, in_maps, core_ids=list(range(NCORES)))

    y = np.empty(NCORES * NBLK * U, np.float32)
    for c in range(NCORES):
        a = res.results[c]["y"].astype(np.float32)   # [128, 2304]
        for b in range(NBLK):
            t = a[:, 768 * b:768 * (b + 1)].reshape(128, 3, 256)
            t = t.transpose(1, 0, 2)                  # [tile, p, n1]
            seg = np.concatenate([t[2][32:], t[0], t[1]], axis=0)
            g = NBLK * c + b
            y[U * g:U * (g + 1)] = seg.reshape(-1)
    return y[:NOUT]


# revision 86
# speedup vs baseline: 1.0168x; 1.0168x over previous
"""Trainium2 Bass kernel: full 1-D convolution (2,097,152 samples x 32,000-tap
RIR) + peak-normalize, via FFT overlap-save, distributed over 8 NeuronCores.

Each core processes 3 overlap-save blocks of M = 131072 = D1*D2 (D1=256,
D2=512).  A block is convolved as y = IDFT(DFT(x) * Hhat) with both DFTs done
as two-stage matmul FFTs on the PE array (n = n1 + 256*n2, k = k2 + 512*k1):

  s1: S[k2,n1]   = sum_n2 F512[n2,k2] x[n1+256*n2]        (real input, k2<=256)
  t1: B = S^T * exp(-2pi i n1 k2 / M)                      (DVE twiddle)
  s3: X[k1,k2]   = sum_n1 F256[n1,k1] B[n1,k2]             (k1 < 128 half)
  pw: Y = X * Hhat[k1,k2]                                  (DVE pointwise)
  s4: D[k2,n1]   = sum_k1 Y[k1,k2] G256[n1,k1]             (k2 in 1..256 ONLY)
  t2: E = D * exp(+2pi i n1 k2 / M)/1024                   (DVE twiddle, half)
  s6: y[n2,n1]   = sum_{k2=1..256} 2 Re(G512[n2,k2] E)     (doubled weights)

The inverse side exploits E[n1,.] being the DFT of the real row y[n1,.]:
E is Hermitian along k2, so S4/S6 run on k2 in [1,256] with doubled S6
weights.  The k1 in [128,256) completion uses Yf[j'] = Y[:, 511-j'] flipped
copies + sign-folded tables.  The DC column (k2=0) is computed by two tiny
[1,256] matmuls and packed into the (otherwise zero-weighted) imag plane of
the k2=256 row; the S6 weight row there is 1s.  The (k1=128, k2=0) M/2 bin
is dropped (same as the f32 reference's error floor; ~1e-3 relative).

Data path fp16 (PE matmuls full rate, DVE tensor ops 2x); PSUM fp32.
Peak-normalize: per-block |max| reduce, AllReduce(max) across 8 cores,
scale by 1/max(m, 1) before the store.
"""
import numpy as np

M = 131072
D1 = 256                  # n1 / k1 extent
D2 = 512                  # n2 / k2 extent
KLEN = 32_000             # RIR taps
N = 2_097_152             # signal samples
NOUT = N + KLEN - 1       # full-convolution output length
U = 90112                 # kept samples per block (352 rows of 256)
DROWS = 160               # discarded n2-rows per block (160*256 = 40960 >= K-1)
KROWS = 352               # kept n2-rows per block
NBLK = 3                  # blocks per core
NCORES = 8
HSCALE = 1024.0           # folded into Hhat; 1/HSCALE folded into tw2/dc

_NC_CACHE = None
STAGE_MARKS = []


def _build_nc(collective=True, passes=1, debug_taps=False):
    import concourse.bacc as bacc
    import concourse.bass as bass
    import concourse.mybir as mybir
    from concourse import tile

    f32 = mybir.dt.float32
    f16 = mybir.dt.float16
    OP = mybir.AluOpType
    AX = mybir.AxisListType.X

    nc = bacc.Bacc("TRN2", target_bir_lowering=False, debug=False,
                   num_devices=NCORES)

    x_in = nc.dram_tensor("x", [NBLK, 128, 4 * D1], f16, kind="ExternalInput")
    wf512 = nc.dram_tensor("wf512", [128, 2056], f16, kind="ExternalInput")
    wf256 = nc.dram_tensor("wf256", [128, 768], f16, kind="ExternalInput")
    wg256 = nc.dram_tensor("wg256", [128, 1536], f16, kind="ExternalInput")
    wg512 = nc.dram_tensor("wg512", [128, 2048], f16, kind="ExternalInput")
    tw1_in = nc.dram_tensor("tw1", [128, 1028], f16, kind="ExternalInput")
    tw2_in = nc.dram_tensor("tw2", [128, 1024], f16, kind="ExternalInput")
    hh_in = nc.dram_tensor("hh", [128, 1024], f16, kind="ExternalInput")
    dc_in = nc.dram_tensor("dc", [128, 512], f16, kind="ExternalInput")
    wf256c_in = nc.dram_tensor("wf256c", [128, 768], f16,
                               kind="ExternalInput")
    y_out = nc.dram_tensor("y", [128, 9 * D1], f16, kind="ExternalOutput")
    if debug_taps:
        dbg = {n: nc.dram_tensor(f"dbg_{n}", [128, w], f16,
                                 kind="ExternalOutput")
               for n, w in (("ssw0", 514), ("ssw1", 514),
                            ("b0", 514), ("b1", 514),
                            ("yh0", 512), ("yh1", 512),
                            ("yf0", 256), ("yf1", 256),
                            ("dt", 1024), ("et", 1024), ("ysb", 768))}

    with tile.TileContext(nc) as tc:
        with (
            tc.tile_pool(name="wpool", bufs=1) as wpool,
            tc.tile_pool(name="data", bufs=2) as dpool,
            tc.tile_pool(name="ypool", bufs=1) as ypool,
            tc.tile_pool(name="ps", bufs=8, space="PSUM") as ps_pool,
            tc.tile_pool(name="dram", bufs=1, space="DRAM") as dram_pool,
        ):
            # ---- weight SBUF tiles ----
            wf512m = wpool.tile([128, 2056], f16, name="wf512m")
            wg512m = wpool.tile([128, 2048], f16, name="wg512m")
            wf256m = wpool.tile([128, 768], f16, name="wf256m")
            wg256m = wpool.tile([128, 1536], f16, name="wg256m")
            tw1m = wpool.tile([128, 1028], f16, name="tw1m")
            tw2m = wpool.tile([128, 1024], f16, name="tw2m")
            hhm = wpool.tile([128, 1024], f16, name="hhm")
            dcm = wpool.tile([128, 512], f16, name="dcm")
            wf256cm = wpool.tile([128, 768], f16, name="wf256cm")
            # plane views
            tw1_sb = [tw1m[:, 514 * p:514 * (p + 1)] for p in range(2)]
            hh_sb = [hhm[:, 512 * p:512 * (p + 1)] for p in range(2)]
            wf256_sb = [wf256m[:, 256 * w:256 * (w + 1)] for w in range(3)]
            wf256c_sb = [wf256cm[:, 256 * w:256 * (w + 1)] for w in range(3)]
            # s4 tables: [C, S, mS, Cp, mSp, mCp]
            g4 = [wg256m[:, 256 * w:256 * (w + 1)] for w in range(6)]
            # s6 tables: [WrA, WiA, WrB, WiB] each 512 wide
            wg512_sb = [wg512m[:, 512 * w:512 * (w + 1)] for w in range(4)]

            y_all = ypool.tile([128, 9 * D1], f16, name="y_all")
            y_sb = [y_all[:, 3 * D1 * b:3 * D1 * (b + 1)]
                    for b in range(NBLK)]
            y16 = ypool.tile([128, 9 * D1], f16, name="y16")
            mx = ypool.tile([128, 3 * NBLK], f32, name="mx")
            am = ypool.tile([128, 1], f32, name="am")
            gm = ypool.tile([128, 1], f32, name="gm")
            scb = ypool.tile([128, 1], f32, name="scb")

            # warm the PE (p-state ramp) during the initial DMA wait
            warm = ypool.tile([128, 512], f16, name="warm")
            nc.gpsimd.memset(warm[:], 0.0)
            wps = ps_pool.tile([128, 512], f32, name="wps", tag="ps")
            for _ in range(4):
                nc.tensor.matmul(wps[:], warm[:, :128], warm[:],
                                 start=True, stop=True)

            # ---- input / weight DMAs ordered by first use ----
            x_sb = [dpool.tile([128, 4 * D1], f16, name=f"x{b}", tag="x",
                               bufs=3) for b in range(NBLK)]
            nc.sync.dma_start(x_sb[0][:], x_in[0])
            for c in range(4):
                nc.sync.dma_start(wf512m[:, 514 * c:514 * (c + 1)],
                                  wf512[:, 514 * c:514 * (c + 1)])
                if c == 0:
                    nc.sync.dma_start(x_sb[1][:], x_in[1])
            nc.sync.dma_start(x_sb[2][:], x_in[2])
            nc.sync.dma_start(tw1m[:], tw1_in[:, :])
            nc.sync.dma_start(wf256m[:], wf256[:, :])
            nc.sync.dma_start(wf256cm[:], wf256c_in[:, :])
            nc.sync.dma_start(hhm[:], hh_in[:, :])
            nc.sync.dma_start(wg256m[:], wg256[:, :])
            nc.sync.dma_start(dcm[:], dc_in[:, :])
            nc.sync.dma_start(tw2m[:], tw2_in[:, :])
            nc.sync.dma_start(wg512m[:], wg512[:, :])

            def cmul(dst_r, dst_i, sr, si, tr, ti, tm, npool=1):
                # z = s * t via 4 mults + add/sub.  One product (m3) on the
                # Pool engine, issued first; the real plane (dst_r) is
                # produced early so downstream matmul chains that read it
                # first can start ~0.6us sooner.
                m0, m1, m2, m3 = tm
                if npool:
                    nc.gpsimd.tensor_mul(m3, si, tr)
                nc.vector.tensor_mul(m0, sr, tr)
                nc.vector.tensor_mul(m1, si, ti)
                nc.vector.tensor_sub(dst_r, m0, m1)
                nc.vector.tensor_mul(m2, sr, ti)
                if not npool:
                    nc.vector.tensor_mul(m3, si, tr)
                nc.vector.tensor_add(dst_i, m2, m3)

            def one_pass():
                nc.gpsimd.memset(mx[:], 0.0)

                # per-block SBUF tiles (tag rotation = double buffering)
                Ssw, B_sb, X_sb, Yh_sb, Yf_sb = [], [], [], [], []
                DtR, EtR, tmps, dcc = [], [], [], []

                def half(t, p):
                    # plane view of the [DrA DiA DrB DiB] layout: 2-run AP
                    return t[:, 0:1024].rearrange(
                        "p (c w) -> p c w", c=2)[:, :, 256 * p:256 * (p + 1)]
                for b in range(NBLK):
                    # bufs=3: the stage-grouped schedule keeps all three
                    # blocks' tiles live simultaneously
                    Ssw.append([dpool.tile([128, 514], f16,
                                name=f"Ssw{b}_{p}", tag=f"Ssw{p}", bufs=3)
                                for p in range(2)])
                    B_sb.append([dpool.tile([128, 514], f16,
                                 name=f"B{b}_{p}", tag=f"B{p}", bufs=3)
                                 for p in range(2)])
                    X_sb.append([dpool.tile([128, D2], f16,
                                 name=f"X{b}_{p}", tag=f"X{p}", bufs=3)
                                 for p in range(2)])
                    Yh_sb.append([dpool.tile([128, D2], f16,
                                  name=f"Yh{b}_{p}", tag=f"Yh{p}", bufs=3)
                                  for p in range(2)])
                    Yf_sb.append([dpool.tile([128, 256], f16,
                                  name=f"Yf{b}_{p}", tag=f"Yf{p}", bufs=3)
                                  for p in range(2)])
                    # layout [DrA(256) DiA DrB DiB]; plane views are 2-run APs
                    DtR.append(dpool.tile([128, 1024], f16, name=f"Dt{b}",
                                          tag="Dt", bufs=3))
                    EtR.append(dpool.tile([128, 1024], f16, name=f"Et{b}",
                                          tag="Et", bufs=3))

                    tmps.append({st: [dpool.tile([128, w], f16,
                                  name=f"tmp{st}{b}_{i}", tag=f"tmp{st}{i}",
                                  bufs=3)
                                  for i in range(4)]
                                 for st, w in (("1", 514), ("3", 512),
                                               ("4", 512))})

                def st_S1(b):
                    # S^T[n1,k2] = x^T @ F512 (x chunks stationary); real
                    # input.  Chunk-major for b=0 so compute starts as soon
                    # as the first weight chunk lands.
                    pss = [ps_pool.tile([128, 512], f32, tag="ps",
                                        name=f"s1_{b}_{n1h}_{p}")
                           for n1h in range(2) for p in range(2)]
                    def mm(n1h, p, c):
                        nc.tensor.matmul(
                            pss[2 * n1h + p][:, 0:257],
                            x_sb[b][:, D1 * c + 128 * n1h:
                                    D1 * c + 128 * (n1h + 1)],
                            wf512m[:, 514 * c + 257 * p:
                                   514 * c + 257 * (p + 1)],
                            start=(c == 0), stop=(c == 3))
                    if b == 0:
                        for c in range(4):
                            for n1h in range(2):
                                for p in range(2):
                                    mm(n1h, p, c)
                    else:
                        for n1h in range(2):
                            for p in range(2):
                                for c in range(4):
                                    mm(n1h, p, c)
                    for n1h in range(2):
                        for p in range(2):
                            nc.scalar.copy(
                                Ssw[b][p][:, 257 * n1h:257 * (n1h + 1)],
                                pss[2 * n1h + p][:, 0:257])
                    # twiddle: B = S^T * tw1.  Block 0 (thin PE cover)
                    # does it per-half so S3_0 can start on half 0 early.
                    if b == 0:
                        for hh_ in range(2):
                            sl = slice(257 * hh_, 257 * (hh_ + 1))
                            cmul(B_sb[b][0][:, sl], B_sb[b][1][:, sl],
                                 Ssw[b][0][:, sl], Ssw[b][1][:, sl],
                                 tw1_sb[0][:, sl], tw1_sb[1][:, sl],
                                 [t[:, sl] for t in tmps[b]["1"]], npool=1)
                    else:
                        cmul(B_sb[b][0][:], B_sb[b][1][:],
                             Ssw[b][0][:], Ssw[b][1][:],
                             tw1_sb[0][:], tw1_sb[1][:],
                             [t[:] for t in tmps[b]["1"]], npool=1)

                def st_S3(b):
                    # X[k1,k2] for k1 < 128 (Hermitian supplies the rest);
                    # then Y = X * Hhat; then flipped copies Yf[j]=Y[511-j].
                    for pout in range(2):
                        terms = ([(0, 0), (2, 1)] if pout == 0
                                 else [(1, 0), (0, 1)])
                        fterms = ([(0, 0), (1, 1)] if pout == 0
                                  else [(1, 0), (2, 1)])
                        ps = ps_pool.tile([128, 512], f32, tag="ps",
                                          name=f"s3_{b}_{pout}")
                        seq = [(w, sp, c) for c in range(2)
                               for (w, sp) in terms]
                        for i, (w, sp, c) in enumerate(seq):
                            nc.tensor.matmul(
                                ps[:, 0:256],
                                wf256_sb[w][:, 128 * c:128 * (c + 1)],
                                B_sb[b][sp][:, 257 * c:257 * c + 256],
                                start=(i == 0), stop=(i == 3))
                        # out cols 256:512 = W_c-weighted conj-flip
                        seq = [(w, sp, c) for c in range(2)
                               for (w, sp) in fterms]
                        for i, (w, sp, c) in enumerate(seq):
                            nc.tensor.matmul(
                                ps[:, 256:512],
                                wf256c_sb[w][:, 128 * c:128 * (c + 1)],
                                B_sb[b][sp][:, 257 * c + 256:
                                            257 * c:-1],
                                start=(i == 0), stop=(i == 3))
                        nc.scalar.copy(X_sb[b][pout][:], ps[:])
                    cmul(Yh_sb[b][0][:], Yh_sb[b][1][:],
                         X_sb[b][0][:], X_sb[b][1][:],
                         hh_sb[0][:], hh_sb[1][:],
                         [t[:] for t in tmps[b]["3"]], npool=1)
                    # flipped copies for the k1-completion reads in S4;
                    # the last block's go on Pool (idle late) to keep Act's
                    # endgame queue clear for the y evacuations
                    yf_eng = (nc.gpsimd.tensor_copy if b == NBLK - 1
                              else nc.scalar.copy)
                    yf_eng(Yf_sb[b][0][:], Yh_sb[b][0][:, 511:255:-1])
                    yf_eng(Yf_sb[b][1][:], Yh_sb[b][1][:, 511:255:-1])

                def st_S4(b):
                    # D[k2,n1] for k2 in 1..256 (two 128-chunks), plus the
                    # DC column via two [1,256] matmuls; then E = D * tw2.
                    yr, yi = Yh_sb[b][0], Yh_sb[b][1]
                    yfr, yfi = Yf_sb[b][0], Yf_sb[b][1]
                    pcs = []
                    for ci, (dlo, flo) in enumerate(((1, 0), (129, 128))):
                        ps = ps_pool.tile([128, 512], f32, tag="ps",
                                          name=f"s4_{b}_{ci}")
                        pcs.append(ps)
                        dsl = slice(dlo, dlo + 128)
                        fsl = slice(flo, flo + 128)
                        # Dr chain: Yr*C + Yi*(-S) + Yfr*C' + Yfi*(-S')
                        # Di chain: Yr*S + Yi*C + Yfr*(-S') + Yfi*(-C')
                        chains = [(0, [(yr, dsl, 0), (yi, dsl, 2),
                                       (yfr, fsl, 3), (yfi, fsl, 4)]),
                                  (1, [(yr, dsl, 1), (yi, dsl, 0),
                                       (yfr, fsl, 4), (yfi, fsl, 5)])]
                        for pout, chain in chains:
                            reg = ps[:, 256 * pout:256 * (pout + 1)]
                            for i, (src, sl, w) in enumerate(chain):
                                nc.tensor.matmul(
                                    reg, src[:, sl], g4[w][:],
                                    start=(i == 0), stop=(i == 3))
                    # evacuate chunk psums into [DrA DiA DrB DiB]
                    nc.scalar.copy(DtR[b][:, 0:512], pcs[0][:])
                    nc.scalar.copy(DtR[b][:, 512:1024], pcs[1][:])
                    # twiddle (single cmul over both chunks via 2-run APs)
                    cmul(half(EtR[b], 0), half(EtR[b], 1),
                         half(DtR[b], 0), half(DtR[b], 1),
                         half(tw2m, 0), half(tw2m, 1),
                         [t[:] for t in tmps[b]["4"]], npool=1)

                def st_S6(b):
                    # y[n2,n1] = sum over k2-chunks of doubled-weight real
                    # parts; n2h=0 all-discard: skip.  n2h order 1,2,3 with
                    # the |max| reduce issued right after each chain so the
                    # final reduce in the tail is a single 256-wide op.
                    if b == NBLK - 1:
                        # separate banks: the per-chain psum reduces must
                        # not block the next chain's matmuls (bank-granular
                        # read/write serialization)
                        ps_y = [ps_pool.tile([128, 512], f32, tag="ps",
                                             name=f"y{n}_{b}")
                                for n in range(3)]
                        regmap = {1: ps_y[0][:, 0:256],
                                  2: ps_y[1][:, 0:256],
                                  3: ps_y[2][:, 0:256]}
                    else:
                        ps_y = [ps_pool.tile([128, 512], f32, tag="ps",
                                             name=f"y23_{b}"),
                                ps_pool.tile([128, 512], f32, tag="ps",
                                             name=f"y1_{b}")]
                        regmap = {2: ps_y[0][:, 0:256],
                                  3: ps_y[0][:, 256:512],
                                  1: ps_y[1][:, 0:256]}
                    yr, yi = Yh_sb[b][0], Yh_sb[b][1]
                    ybc = [yr[:, 0:1].broadcast_to((128, 128)),
                           yi[:, 0:1].broadcast_to((128, 128))]
                    for n2h in (1, 2, 3):
                        reg = regmap[n2h]
                        # DC rows (replicated broadcast-lhsT matmuls), then
                        # (weight-table, Et col range): WrA*DrA + WiA*DiA
                        # + WrB*DrB + WiB*DiB
                        nc.tensor.matmul(reg, ybc[0], dcm[:, 0:256],
                                         start=True, stop=False)
                        nc.tensor.matmul(reg, ybc[1], dcm[:, 256:512],
                                         start=False, stop=False)
                        seq = [(0, 0), (2, 512), (1, 256), (3, 768)]
                        for i, (w, off) in enumerate(seq):
                            nc.tensor.matmul(
                                reg,
                                wg512_sb[w][:, 128 * n2h:128 * (n2h + 1)],
                                EtR[b][:, off:off + 256],
                                start=False, stop=(i == 3))
                        if b == NBLK - 1:
                            # last block: reduce each chain from psum
                            # immediately so the tail reduce is short
                            if n2h == 1:
                                # copy early, zero discard rows (free
                                # memset), one reduce from fp16 staging
                                nc.scalar.copy(y_sb[b][:, 512:768],
                                               regmap[1])
                                nc.vector.memset(y_sb[b][0:32, 512:768],
                                                 0.0)
                                nc.vector.tensor_reduce(
                                    mx[:, 3 * b:3 * b + 1],
                                    y_sb[b][:, 512:768],
                                    axis=AX, op=OP.max,
                                    apply_absolute_value=True)
                            else:
                                nc.vector.tensor_reduce(
                                    mx[:, 3 * b + n2h - 1:3 * b + n2h],
                                    regmap[n2h], axis=AX, op=OP.max,
                                    apply_absolute_value=True)
                        if n2h == 1 and b < NBLK - 1:
                            nc.scalar.copy(y_sb[b][:, 512:768],
                                           regmap[1])
                    if b == NBLK - 1:
                        nc.scalar.copy(y_sb[b][:, 0:256], regmap[2])
                        nc.scalar.copy(y_sb[b][:, 256:512], regmap[3])
                    else:
                        nc.scalar.copy(y_sb[b][:, 0:512], ps_y[0][:])
                    if b < NBLK - 1:
                        # earlier blocks: zero the discarded rows (memset is
                        # free in the cost model) and reduce from the fp16
                        # staging in two halves (finer DVE slots, keeps the
                        # tail backlog short)
                        nc.vector.memset(y_sb[b][0:32, 512:768], 0.0)
                        nc.vector.tensor_reduce(
                            mx[:, 3 * b:3 * b + 1], y_sb[b][:, 0:384],
                            axis=AX, op=OP.max, apply_absolute_value=True)
                        nc.vector.tensor_reduce(
                            mx[:, 3 * b + 1:3 * b + 2], y_sb[b][:, 384:768],
                            axis=AX, op=OP.max, apply_absolute_value=True)

                stage_fn = {"S1": st_S1, "S3": st_S3,
                            "S4": st_S4, "S6": st_S6}
                schedule = [("S1", 0), ("S1", 1), ("S1", 2),
                            ("S3", 0), ("S3", 1), ("S3", 2),
                            ("S4", 0), ("S4", 1), ("S6", 0),
                            ("S4", 2), ("S6", 1), ("S6", 2)]
                for st, b in schedule:
                    STAGE_MARKS.append((st, b, nc.next_id()))
                    stage_fn[st](b)
                    if debug_taps and (st, b) == ("S1", 0):
                        for p in range(2):
                            nc.sync.dma_start(dbg[f"ssw{p}"][:, :],
                                              Ssw[0][p][:])
                            nc.sync.dma_start(dbg[f"b{p}"][:, :],
                                              B_sb[0][p][:])
                    if debug_taps and (st, b) == ("S3", 0):
                        for p in range(2):
                            nc.sync.dma_start(dbg[f"yh{p}"][:, :],
                                              Yh_sb[0][p][:])
                            nc.sync.dma_start(dbg[f"yf{p}"][:, :],
                                              Yf_sb[0][p][:])
                    if debug_taps and (st, b) == ("S4", 0):
                        nc.sync.dma_start(dbg["dt"][:, :], DtR[0][:])
                        nc.sync.dma_start(dbg["et"][:, :], EtR[0][:])
                    if debug_taps and (st, b) == ("S6", 0):
                        nc.sync.dma_start(dbg["ysb"][:, :], y_sb[0][:])
                STAGE_MARKS.append(("END", 0, nc.next_id()))

                # ---- global max, scale, store ----
                nc.vector.tensor_reduce(am[:], mx[:], axis=AX, op=OP.max)
                nc.gpsimd.partition_all_reduce(
                    gm[:], am[:], 128, bass.bass_isa.ReduceOp.max)
                if collective:
                    cc_in = dram_pool.tile([128, 1], f32, name="cc_in")
                    cc_out = dram_pool.tile([128, 1], f32, name="cc_out",
                                            addr_space="Shared")
                    nc.sync.dma_start(cc_in[:], gm[:])
                    nc.gpsimd.collective_compute(
                        "AllReduce", OP.max,
                        replica_groups=[list(range(NCORES))],
                        ins=[cc_in[:].opt()], outs=[cc_out[:].opt()])
                    nc.sync.dma_start(scb[:], cc_out[:])
                    nc.vector.tensor_scalar_max(scb[:], scb[:], 1.0)
                else:
                    nc.vector.tensor_scalar_max(scb[:], gm[:], 1.0)
                nc.vector.reciprocal(scb[:], scb[:])
                # scale + store in two chunks on two DMA queues: blocks 0-1
                # are staged long before the last block, so their scale and
                # store overlap the last block's copy tail
                nc.vector.tensor_scalar_mul(y16[:, 0:1536],
                                            y_all[:, 0:1536], scb[:, 0:1])
                nc.sync.dma_start(y_out[:, 0:1536], y16[:, 0:1536])
                nc.vector.tensor_scalar_mul(y16[:, 1536:2304],
                                            y_all[:, 1536:2304], scb[:, 0:1])
                nc.gpsimd.dma_start(y_out[:, 1536:2304], y16[:, 1536:2304])

            for _ in range(passes):
                one_pass()

    nc.compile()
    return nc


def _chunk(a):
    """[R, C] -> [128, (R/128)*C]: partition-chunked SBUF layout."""
    r, c = a.shape
    return np.ascontiguousarray(
        a.reshape(r // 128, 128, c).transpose(1, 0, 2).reshape(128, -1))


def _build_tables(h):
    """fp16 host tables; returns dict of np arrays keyed by dram tensor name."""
    n1 = np.arange(D1)
    n2 = np.arange(D2)
    q = np.arange(128)
    F512 = np.exp(-2j * np.pi * np.outer(n2, n2) / D2)
    F256 = np.exp(-2j * np.pi * np.outer(n1, n1) / D1)
    t1 = np.exp(-2j * np.pi * np.outer(n1, n2) / M)
    Wc = F256 * np.exp(-2j * np.pi * n1 / D1)[:, None]
    hp = np.zeros(M)
    hp[:KLEN] = h
    H2 = (np.fft.fft(hp) * (HSCALE / M)).reshape(D1, D2)
    f = np.float16

    # s4 tables [q, n1]
    C = np.cos(2 * np.pi * np.outer(q, n1) / D1)
    S = np.sin(2 * np.pi * np.outer(q, n1) / D1)
    Cp = np.cos(2 * np.pi * np.outer(q + 1, n1) / D1)
    Sp = np.sin(2 * np.pi * np.outer(q + 1, n1) / D1)

    # dc tables [q, n1] (with /HSCALE and the doubled conj-fold; the
    # (k1=128, k2=0) M/2 bin is dropped)
    CD = np.zeros((128, D1))
    SD = np.zeros((128, D1))
    CD[0] = 1.0 / HSCALE
    CD[1:] = 2 * np.cos(2 * np.pi * np.outer(q[1:], n1) / D1) / HSCALE
    SD[1:] = -2 * np.sin(2 * np.pi * np.outer(q[1:], n1) / D1) / HSCALE

    # tw2: k2 = p+1 (chunk A) and p+129 (chunk B); [t2r(A|B) | t2i(A|B)]
    k2v = np.arange(1, 257)
    T2 = np.exp(+2j * np.pi * np.outer(k2v, n1) / M) / HSCALE  # [256, 256]
    # layout [t2rA t2iA t2rB t2iB] matching the [DrA DiA DrB DiB] tiles
    tw2t = np.concatenate([T2.real[:128], T2.imag[:128],
                           T2.real[128:], T2.imag[128:]], axis=1)

    # s6 tables [k2-row p, n2]: WrA/WiA (k2=p+1), WrB/WiB (k2=p+129);
    # row 127 of B is k2=256: weight 1, and WiB row 127 = 1s (DC slot).
    kA = np.outer(q + 1, n2)
    kB = np.outer(q + 129, n2)
    WrA = 2 * np.cos(2 * np.pi * kA / D2)
    WiA = -2 * np.sin(2 * np.pi * kA / D2)
    WrB = 2 * np.cos(2 * np.pi * kB / D2)
    WiB = -2 * np.sin(2 * np.pi * kB / D2)
    WrB[127] /= 2.0
    WiB[127] = 0.0

    # wf512: chunk-major, plane-minor: cols = 514*c + 257*p + k2
    cr, ci = _chunk(F512.real), _chunk(F512.imag)   # [128, 4*512]
    wf512m = np.empty((128, 2056))
    for c in range(4):
        wf512m[:, 514 * c:514 * c + 257] = cr[:, 512 * c:512 * c + 257]
        wf512m[:, 514 * c + 257:514 * (c + 1)] = ci[:, 512 * c:512 * c + 257]
    return {
        "wf512": wf512m.astype(f),
        "wf256": np.concatenate(
            [np.concatenate([_chunk(P)[:, 256 * c:256 * c + 128]
                             for c in range(2)], axis=1)
             for P in (F256.real, F256.imag, -F256.imag)],
            axis=1).astype(f),
        "wg256": np.concatenate([C, S, -S, Cp, -Sp, -Cp], axis=1).astype(f),
        "wg512": np.concatenate([WrA, WiA, WrB, WiB], axis=1).astype(f),
        "tw1": np.concatenate(
            [np.concatenate([_chunk(P)[:, 512 * hh_:512 * hh_ + 257]
                             for hh_ in range(2)], axis=1)
             for P in (t1.real, t1.imag)], axis=1).astype(f),
        "tw2": tw2t.astype(f),
        "hh": np.concatenate(
            [_chunk(P)[:, 0:512] for P in (H2.real, H2.imag)],
            axis=1).astype(f),
        "dc": np.concatenate([CD, SD], axis=1).astype(f),
        "wf256c": np.concatenate(
            [np.concatenate([_chunk(P)[:, 256 * c:256 * c + 128]
                             for c in range(2)], axis=1)
             for P in (Wc.real, Wc.imag, -Wc.real)],
            axis=1).astype(f),
    }


def kernel(data, rir):
    global _NC_CACHE
    from concourse.bass_utils import run_bass_kernel_spmd

    data = np.asarray(data, dtype=np.float32).reshape(-1)
    h = np.asarray(rir, dtype=np.float64).reshape(-1)

    if _NC_CACHE is None:
        _NC_CACHE = _build_nc()
    nc = _NC_CACHE

    tables = _build_tables(h)
    disc = DROWS * D1
    xp = np.zeros(disc + NCORES * NBLK * U + (M - U - disc) + 1, np.float16)
    xp[disc:disc + N] = data.astype(np.float16)
    in_maps = []
    for c in range(NCORES):
        xb = np.stack([
            _chunk(xp[U * (NBLK * c + b):U * (NBLK * c + b) + M]
                   .reshape(D2, D1))
            for b in range(NBLK)])
        in_maps.append({"x": np.ascontiguousarray(xb), **tables})
    res = run_bass_kernel_spmd(nc, in_maps, core_ids=list(range(NCORES)))

    y = np.empty(NCORES * NBLK * U, np.float32)
    for c in range(NCORES):
        a = res.results[c]["y"].astype(np.float32)   # [128, 2304]
        for b in range(NBLK):
            t = a[:, 768 * b:768 * (b + 1)].reshape(128, 3, 256)
            t = t.transpose(1, 0, 2)                  # [tile, p, n1]
            seg = np.concatenate([t[2][32:], t[0], t[1]], axis=0)
            g = NBLK * c + b
            y[U * g:U * (g + 1)] = seg.reshape(-1)
    return y[:NOUT]
